# revision 21
# baseline (speedup 1.0000x reference)
"""Trainium2 Bass kernel for nn_Attention (B=4, P=2048, D=768, H=12, hd=64).

Sharding: 8 cores = 4 batches x 2 half-head-groups (6 heads each).

Schedule design (v2): the Scalar engine's Exp stream (25.2M elems/core,
~1.44us per 1536-col unit, 132 units ~= 190us) is the critical resource.
Everything else is scheduled around keeping it saturated:
  - phase A computes only the qk features needed by the first chunks
    (ft0/ft3) cc-outer across 8 PSUM banks, gated by the input DMA, so
    the first Exp fires ~13us in (baseline: 55us).
  - all remaining PE work (leftover qk projection, v projection, AV,
    output projection) is emitted as fine-grained fill jobs in the PE
    slack between score matmuls, budgeted per exp unit.
  - score slabs are triple-buffered so AV may lag up to 2 chunks.
  - PSUM: 2x[128,1536] score units (6 banks) + shared 2-bank pool for
    all accumulation groups (AV / v / qk / proj), max 2 groups open.
  - v/proj PSUM evacuations run on the Pool (GpSimd) engine; DVE keeps
    the qk evacuations and the softmax normalization chain.
  - chunk order [p0q0 p0q1 p0q2 p1q0 p1q1 p2q0 p2q1 p1q2 p2q2 p0q3
    p1q3 p2q3] completes token blocks early so the output projection
    and its DMA overlap the exp stream; only tb3 remains in the tail.
  - output projection bias (b_proj/2) added on-chip by Pool; host sums
    the two half-head partials per batch and transposes.

Per-core layouts (host-prepared):
  xT   [769, 2048] bf16  rows 0..767 = x[b].T, row 768 = ones
  wqk  [768, 768]  bf16  [c, feat]; feat tiles q(01) q(23) q(45) k(01) k(23) k(45)
  wv   [769, 390]  bf16  [c(+bias/ones row), 6 heads x (64 v-dims, ones-col)]
  wp   [384, 768]  bf16  [feat (6 heads x 64), out-features]
  bqk  [128, 6]    f32   per-partition bias per qk feature tile
  bp   [128, 6]    f32   b_proj / 2 per out-feature tile
Output:
  yT   [768, 2048] f32   partial (pre pair-sum) transposed projection
"""

import sys
from collections import deque

import numpy as np

if "/opt/trn_rl_repo" not in sys.path:
    sys.path.insert(0, "/opt/trn_rl_repo")

B, P, D = 4, 2048, 768
H, HD = 12, 64
N_CORES = 8
H_LOC = 6
SCALE = HD ** -0.5

CC = 6          # contraction chunks of 128 over D=768
KT = 16         # k-position tiles of 128
PT = 16         # token tiles of 128
TB = 4          # token blocks of 512
VW = H_LOC * (HD + 1)  # 390, per-head wv cols [ones, v(64)]
VG = 128        # per-head vsb block: col0 denom-ones, cols 64:128 v
UNIT = 1536
NBLK = 2 * KT   # 32 512-col score blocks per chunk
TOTAL = NBLK * 512
N_UNITS = (TOTAL + UNIT - 1) // UNIT  # 11 (last = 1024)
PERIOD = 1437   # ns, measured steady exp-unit cadence
MM = 213        # ns, 512-col matmul at 2.4 GHz

# chunk order (pair, qq): early tb completion without needing late ft early
CHUNKS = [(0, 0), (0, 1), (0, 2), (1, 0), (1, 1), (2, 0), (2, 1),
          (1, 2), (2, 2), (0, 3), (1, 3), (2, 3)]

_PROG = None


def _build_program():
    import concourse.mybir as mybir
    import concourse.tile as tile
    from concourse import bacc

    f32 = mybir.dt.float32
    bf16 = mybir.dt.bfloat16
    AF = mybir.ActivationFunctionType

    nc = bacc.Bacc("TRN2")

    xT = nc.declare_dram_parameter("xT", [769, 2048], bf16, isOutput=False)
    wqk = nc.declare_dram_parameter("wqk", [768, 768], bf16, isOutput=False)
    wv = nc.declare_dram_parameter("wv", [769, VW], bf16, isOutput=False)
    wp = nc.declare_dram_parameter("wp", [384, 768], bf16, isOutput=False)
    bqk = nc.declare_dram_parameter("bqk", [128, 6], f32, isOutput=False)
    bp = nc.declare_dram_parameter("bp", [128, 6], f32, isOutput=False)
    yT = nc.declare_dram_parameter("yT", [768, 2048], f32, isOutput=True)

    with tile.TileContext(nc) as tc:
        with (
            tc.tile_pool(name="persist", bufs=1) as persist,
            tc.tile_pool(name="slabs", bufs=2) as slabs,
            tc.tile_pool(name="norm", bufs=3) as norm,
            tc.tile_pool(name="ysl", bufs=2) as yslp,
            tc.tile_pool(name="drs", bufs=4, space="DRAM") as drs,
        ):
            qkt = persist.tile([128, 6, 2048], bf16, tag="qkt")
            vsb = persist.tile([128, PT, H_LOC * VG], bf16, tag="vsb")
            nc.gpsimd.memset(
                vsb.rearrange("p a (h g) -> p a h g", g=VG)[:, :, :, 1:64], 0.0)
            otsb = persist.tile([128, 3, 2048], bf16, tag="otsb")
            bqk_sb = persist.tile([128, 6], f32, tag="bqk_sb")
            bp_sb = persist.tile([128, 6], f32, tag="bp_sb")
            wp_sb = persist.tile([128, 3, 768], bf16, tag="wp_sb")
            xts = [
                persist.tile([128 if i < CC else 1, 2048], bf16,
                             tag=f"xt{i}", name=f"xt{i}")
                for i in range(7)
            ]
            wqk_sbs = [
                persist.tile([128, 768], bf16, tag=f"wqk{i}", name=f"wqk{i}")
                for i in range(CC)
            ]
            wv_sbs = [
                persist.tile([128 if i < CC else 1, VW], bf16,
                             tag=f"wv{i}", name=f"wv{i}")
                for i in range(7)
            ]

            # ---- input DMA, priority ordered. Per-queue rings are FIFO;
            # rings run concurrently and share HBM bandwidth, so only the
            # phase-A-critical bytes go first: xT alone on the sync ring,
            # the ft0/ft3 columns of wqk (+biases) on the scalar ring.
            # wv rides the sync ring behind xT; the remaining wqk columns
            # and wp trail at the end of both rings.
            for ccx in range(CC):
                nc.sync.dma_start(out=xts[ccx],
                                  in_=xT[ccx * 128:(ccx + 1) * 128, :])
                for ft in (3, 0):
                    nc.scalar.dma_start(
                        out=wqk_sbs[ccx][:, ft * 128:(ft + 1) * 128],
                        in_=wqk[ccx * 128:(ccx + 1) * 128,
                                ft * 128:(ft + 1) * 128])
            nc.sync.dma_start(out=xts[6], in_=xT[768:769, :])
            nc.scalar.dma_start(out=bqk_sb, in_=bqk[:, :])
            for ccx in range(CC):
                nc.sync.dma_start(out=wv_sbs[ccx],
                                  in_=wv[ccx * 128:(ccx + 1) * 128, :])
            nc.sync.dma_start(out=wv_sbs[6], in_=wv[768:769, :])
            for ccx in range(CC):
                for ft in (4, 1, 5, 2):
                    nc.scalar.dma_start(
                        out=wqk_sbs[ccx][:, ft * 128:(ft + 1) * 128],
                        in_=wqk[ccx * 128:(ccx + 1) * 128,
                                ft * 128:(ft + 1) * 128])
            for fc in range(3):
                nc.sync.dma_start(out=wp_sb[:, fc, :],
                                  in_=wp[fc * 128:(fc + 1) * 128, :])
            nc.scalar.dma_start(out=bp_sb, in_=bp[:, :])

            # pre-warm the exp ACT table during the DMA lead
            warmup = norm.tile([1, 1], f32, tag="warmup", bufs=1)
            nc.vector.memset(warmup, 0.0)
            nc.scalar.activation(out=warmup, in_=warmup, func=AF.Exp)

            # ===== phase A: qk projection for ft0 (q pair0) + ft3 (k pair0)
            # cc-outer over 8 PSUM groups, gated by per-cc DMA arrival.
            psA_ctx = tc.tile_pool(name="psA", bufs=8, space="PSUM")
            psA = psA_ctx.__enter__()
            qpA = {}
            for ft in (3, 0):
                for tb in range(TB):
                    qpA[(ft, tb)] = psA.tile([128, 512], f32, tag="qpA",
                                             name=f"qpA{ft}_{tb}")
            for ccx in range(CC):
                for ft in (3, 0):
                    for tb in range(TB):
                        nc.tensor.matmul(
                            qpA[(ft, tb)],
                            wqk_sbs[ccx][:, ft * 128:(ft + 1) * 128],
                            xts[ccx][:, tb * 512:(tb + 1) * 512],
                            start=(ccx == 0),
                            stop=(ccx == CC - 1),
                        )
            # evac order: what chunk-0's first exp units need first
            for ft, tb in ((3, 0), (0, 0), (3, 1), (3, 2), (3, 3),
                           (0, 1), (0, 2), (0, 3)):
                nc.vector.tensor_scalar_add(
                    out=qkt[:, ft, tb * 512:(tb + 1) * 512],
                    in0=qpA[(ft, tb)],
                    scalar1=bqk_sb[:, ft:ft + 1],
                )
            psA_ctx.__exit__(None, None, None)

            # ===== phase B: exp-gated chunk pipeline with fill scheduler
            with (
                tc.tile_pool(name="psum_s", bufs=2, space="PSUM") as psum_s,
                tc.tile_pool(name="psum_w", bufs=2, space="PSUM") as psum_w,
            ):
                # ---------- fill-job machinery ----------
                # a group = generator yielding (cost_ns, emit_fn) steps;
                # at most 2 groups hold psum_w tiles concurrently.
                # groups are picked by deadline (chunk index of first use).
                import heapq

                state = {
                    "budget": 0.0,
                    "unit": 0,
                    "v_evacs": 0,
                    "active": [],      # [(deadline, needs_v, gen), ...] <= 2
                    "queue": [],       # heap of (deadline, seq, needs_v, gen)
                    "seq": 0,
                    "pending": [],     # (ready_unit, emit_fn)
                    "norm_count": {},  # qq -> heads normalized
                    "proj_pushed": set(),
                    "chunk": 0,
                }

                def v_done():
                    return state["v_evacs"] >= PT

                def push(deadline, gen, needs_v=False):
                    heapq.heappush(
                        state["queue"],
                        (deadline, state["seq"], needs_v, gen))
                    state["seq"] += 1

                def v_group(pt):
                    vp = psum_w.tile([128, 512], f32, tag="grp",
                                     name=f"vp{pt}")
                    for ccx in range(7):
                        kk = 128 if ccx < CC else 1

                        def mm(ccx=ccx, kk=kk, vp=vp):
                            nc.tensor.matmul(
                                vp[:, 0:VW],
                                xts[ccx][0:kk, pt * 128:(pt + 1) * 128],
                                wv_sbs[ccx][0:kk, :],
                                start=(ccx == 0),
                                stop=(ccx == 6),
                            )
                        yield (162 if ccx < CC else 30, mm)

                    def evac(vp=vp):
                        vpv = vp[:, 0:VW].rearrange("p (h c) -> p h c", c=65)
                        vdst = vsb.rearrange(
                            "p a (h g) -> p a h g", g=VG)[:, pt]
                        nc.vector.tensor_copy(out=vdst[:, :, 0:1],
                                              in_=vpv[:, :, 0:1])
                        nc.vector.tensor_copy(out=vdst[:, :, 64:128],
                                              in_=vpv[:, :, 1:65])
                        state["v_evacs"] += 1
                    yield (0, evac)

                def qk_group(ft, tb):
                    qp = psum_w.tile([128, 512], f32, tag="grp",
                                     name=f"qp{ft}_{tb}")
                    for ccx in range(CC):
                        def mm(ccx=ccx, qp=qp):
                            nc.tensor.matmul(
                                qp,
                                wqk_sbs[ccx][:, ft * 128:(ft + 1) * 128],
                                xts[ccx][:, tb * 512:(tb + 1) * 512],
                                start=(ccx == 0),
                                stop=(ccx == CC - 1),
                            )
                        yield (MM, mm)

                    def evac(qp=qp):
                        nc.vector.tensor_scalar_add(
                            out=qkt[:, ft, tb * 512:(tb + 1) * 512],
                            in0=qp,
                            scalar1=bqk_sb[:, ft:ft + 1],
                        )
                    yield (0, evac)

                def norm_finish(ph, qq, osb, rb):
                    """emitted >=2 units after the AV group ends"""
                    pb = 64 * (ph % 2)
                    nc.vector.tensor_mul(
                        out=otsb[pb:pb + 64, ph // 2,
                                 qq * 512:(qq + 1) * 512],
                        in0=osb[64:128, :],
                        in1=rb[64:128, :],
                    )
                    cnt = state["norm_count"]
                    cnt[qq] = cnt.get(qq, 0) + 1
                    if cnt[qq] == H_LOC:
                        push_proj(qq)

                def av_group(p, qq, hd, slab):
                    ph = 2 * p + hd
                    op = psum_w.tile([128, 512], f32, tag="grp",
                                     name=f"op{ph}_{qq}")
                    for kc in range(KT):
                        def mm(kc=kc, op=op):
                            nc.tensor.matmul(
                                op,
                                vsb[:, kc, ph * VG:(ph + 1) * VG],
                                slab[:, kc * 2 + hd, :],
                                start=(kc == 0),
                                stop=(kc == KT - 1),
                            )
                        yield (MM, mm)

                    def drain(op=op):
                        osb = norm.tile([128, 512], f32, tag="osb")
                        nc.vector.tensor_copy(out=osb, in_=op)
                        rec = norm.tile([1, 512], f32, tag="rec", bufs=2)
                        rsc = norm.tile([1, 512], f32, tag="rsc", bufs=2)
                        nc.vector.reciprocal_approx_accurate(
                            out=rec, in_=osb[0:1, :], scratch=rsc)
                        dsc = drs.tile([1, 512], f32, tag="dsc")
                        nc.sync.dma_start(out=dsc, in_=rec)
                        rb = norm.tile([128, 512], f32, tag="rb", bufs=2)
                        nc.gpsimd.dma_start(
                            out=rb[64:128, :],
                            in_=dsc.partition_broadcast(64))
                        state["pending"].append(
                            (state["unit"] + 2,
                             lambda: norm_finish(ph, qq, osb, rb)))
                    yield (0, drain)

                def proj_group(of, tb):
                    pp = psum_w.tile([128, 512], f32, tag="grp",
                                     name=f"pp{of}_{tb}")
                    for fc in range(3):
                        def mm(fc=fc, pp=pp):
                            nc.tensor.matmul(
                                pp,
                                wp_sb[:, fc, of * 128:(of + 1) * 128],
                                otsb[:, fc, tb * 512:(tb + 1) * 512],
                                start=(fc == 0),
                                stop=(fc == 2),
                            )
                        yield (MM, mm)

                    def evac(pp=pp):
                        ysl = yslp.tile([128, 512], f32, tag="ysl")
                        nc.vector.tensor_scalar_add(
                            out=ysl, in0=pp, scalar1=bp_sb[:, of:of + 1])
                        nc.sync.dma_start(
                            out=yT[of * 128:(of + 1) * 128,
                                   tb * 512:(tb + 1) * 512],
                            in_=ysl,
                        )
                    yield (0, evac)

                def push_proj(tb):
                    if tb in state["proj_pushed"]:
                        return
                    state["proj_pushed"].add(tb)
                    for of in range(6):
                        push(state["chunk"] + 1.1, proj_group(of, tb))

                def refill_active():
                    while len(state["active"]) < 2 and state["queue"]:
                        dl, sq, needs_v, gen = state["queue"][0]
                        if needs_v and not v_done():
                            # find first runnable entry instead
                            runnable = [e for e in state["queue"]
                                        if not e[2]]
                            if not runnable:
                                return
                            entry = min(runnable)
                            state["queue"].remove(entry)
                            heapq.heapify(state["queue"])
                            dl, sq, needs_v, gen = entry
                        else:
                            heapq.heappop(state["queue"])
                        state["active"].append((dl, needs_v, gen))

                def emit_fills(limit=None):
                    emitted = 0
                    refill_active()
                    while state["active"] and state["budget"] > 0:
                        entry = state["active"].pop(0)
                        try:
                            cost, fn = next(entry[2])
                        except StopIteration:
                            refill_active()
                            continue
                        fn()
                        state["budget"] -= cost
                        state["active"].append(entry)
                        emitted += 1
                        if limit is not None and emitted >= limit:
                            break
                        refill_active()

                def force_drain(thr):
                    """fully emit every group whose deadline precedes thr —
                    consumers in chunk ceil(thr) are about to be emitted and
                    Tile only orders by program order."""
                    progress = True
                    while progress:
                        progress = False
                        for entry in list(state["active"]):
                            dl, needs_v, g = entry
                            if dl < thr and (not needs_v or v_done()):
                                state["active"].remove(entry)
                                for cost, fn in g:
                                    fn()
                                    state["budget"] -= cost
                                progress = True
                        while True:
                            cands = [e for e in state["queue"]
                                     if e[0] < thr
                                     and (not e[2] or v_done())]
                            if not cands:
                                break
                            entry = min(cands)
                            state["queue"].remove(entry)
                            heapq.heapify(state["queue"])
                            for cost, fn in entry[3]:
                                fn()
                                state["budget"] -= cost
                            progress = True

                def unit_tick():
                    u = state["unit"]
                    still = []
                    for ready, fn in state["pending"]:
                        if u >= ready:
                            fn()
                        else:
                            still.append((ready, fn))
                    state["pending"] = still
                    state["unit"] = u + 1

                # seed the queue: v projection, then leftover qk features
                # with deadlines = first chunk that reads them
                for pt in range(PT):
                    push(1.5 + 0.02 * pt, v_group(pt))
                first_pair_chunk = {0: 0, 1: 3, 2: 5}
                for ft, tb in ((4, 0), (4, 1), (4, 2), (4, 3),
                               (5, 0), (5, 1), (5, 2), (5, 3)):
                    # k-features: all four k-tiles are read within the
                    # pair's first chunk, so all must precede it
                    dl = first_pair_chunk[ft - 3] - 0.5 + 0.1 * tb
                    push(dl, qk_group(ft, tb))
                for ft in (1, 2):
                    for tb in range(TB):
                        dl = CHUNKS.index((ft, tb)) - 0.4
                        push(dl, qk_group(ft, tb))

                def score_mm(p, qq, sp, g, off):
                    kt, hd = g // 2, g % 2
                    pb = 64 * hd
                    qlo = qq * 512
                    nc.tensor.matmul(
                        sp[:, off:off + 512],
                        qkt[pb:pb + 64, 3 + p, kt * 128:(kt + 1) * 128],
                        qkt[pb:pb + 64, p, qlo:qlo + 512],
                        start=True,
                        stop=True,
                    )

                for ci, (p, qq) in enumerate(CHUNKS):
                    state["chunk"] = ci
                    # everything chunk ci reads must already be emitted
                    force_drain(ci)
                    slab = slabs.tile([128, NBLK, 512], bf16, tag="slab")
                    # PE below full p-state for the first chunks: charge
                    # emitted fill work at its real (slower) rate
                    slow = 1.6 if ci < 2 else (1.2 if ci == 2 else 1.0)
                    for u in range(N_UNITS):
                        unit_tick()
                        width = min(UNIT, TOTAL - u * UNIT)
                        nblk_u = width // 512
                        state["budget"] += (
                            PERIOD - ((nblk_u + 1) // 2) * MM * slow
                        ) / slow
                        if ci > 0 or u >= 4:
                            # chunk 0's first units are input-DMA gated;
                            # a v fill here would stall PE on the wv DMA
                            emit_fills()
                        sp = psum_s.tile([128, UNIT], f32, tag="sp")
                        for j in range(nblk_u):
                            score_mm(p, qq, sp, u * 3 + j, j * 512)
                        nc.scalar.activation(
                            out=slab.rearrange("p a b -> p (a b)")[
                                :, u * UNIT:u * UNIT + width],
                            in_=sp[:, 0:width],
                            func=AF.Exp,
                            scale=SCALE,
                        )
                    # slab complete: queue its AV; must be fully emitted
                    # before chunk ci+2 reuses the slab buffer slot
                    for hd in range(2):
                        push(ci + 1.35, av_group(p, qq, hd, slab),
                             needs_v=True)

                # ---- tail: drain all remaining groups and pendings ----
                state["chunk"] = len(CHUNKS)
                state["budget"] = 1e9
                guard = 0
                while (state["active"] or state["queue"]
                       or state["pending"]):
                    unit_tick()
                    emit_fills(limit=64)
                    guard += 1
                    assert guard < 10000, "tail drain did not converge"

    nc.finalize()
    return nc


def _get_program():
    global _PROG
    if _PROG is None:
        _PROG = _build_program()
    return _PROG


def _prep_core_inputs(x, w_qkv, b_qkv, w_proj, b_proj, core):
    b, half = core // 2, core % 2
    heads = np.arange(H_LOC) + H_LOC * half
    d = np.arange(HD)

    import ml_dtypes
    bft = ml_dtypes.bfloat16
    xT = np.empty((769, 2048), bft)
    xT[:768] = x[b].T.astype(bft)
    xT[768] = 1.0

    # torch reshape quirk: feature (t, d, h) -> row t*768 + d*12 + h
    qk_rows = np.empty(768, np.int64)
    for j in range(3):
        for hp in range(2):
            hh = heads[2 * j + hp]
            base = j * 128 + hp * 64
            qk_rows[base:base + 64] = d * 12 + hh
            qk_rows[384 + base:384 + base + 64] = 768 + d * 12 + hh
    wqk = np.ascontiguousarray(w_qkv[qk_rows].T.astype(bft))
    bqk = np.ascontiguousarray(b_qkv[qk_rows].reshape(6, 128).T)

    # per-head cols: [64 v-dims, ones]; row 768 = bias (+1.0 in ones col)
    wv = np.zeros((769, VW), bft)
    for i in range(H_LOC):
        rows = 1536 + d * 12 + heads[i]
        wv[768, 65 * i] = 1.0
        wv[:768, 65 * i + 1:65 * i + 65] = w_qkv[rows].T.astype(bft)
        wv[768, 65 * i + 1:65 * i + 65] = b_qkv[rows]

    wp = np.empty((384, 768), bft)
    for i in range(H_LOC):
        cols = 64 * heads[i] + d
        wp[64 * i:64 * i + 64] = w_proj[:, cols].T
    bp = np.ascontiguousarray((b_proj * 0.5).reshape(6, 128).T)

    return {
        "xT": xT,
        "wqk": wqk,
        "wv": np.ascontiguousarray(wv),
        "wp": np.ascontiguousarray(wp),
        "bqk": bqk,
        "bp": np.ascontiguousarray(bp),
    }


def _run(inputs, trace=False, **kw):
    from concourse.bass_utils import run_bass_kernel_spmd

    nc = _get_program()
    x = np.asarray(inputs["x"], np.float32)
    w_qkv = np.asarray(inputs["w_qkv"], np.float32)
    b_qkv = np.asarray(inputs["b_qkv"], np.float32)
    w_proj = np.asarray(inputs["w_proj"], np.float32)
    b_proj = np.asarray(inputs["b_proj"], np.float32)

    in_maps = [
        _prep_core_inputs(x, w_qkv, b_qkv, w_proj, b_proj, c)
        for c in range(N_CORES)
    ]
    res = run_bass_kernel_spmd(nc, in_maps, list(range(N_CORES)),
                               trace=trace, **kw)

    out = np.empty((B, P, D), np.float32)
    for b in range(B):
        yt = res.results[2 * b]["yT"] + res.results[2 * b + 1]["yT"]
        out[b] = yt.T
    return out, res


def kernel(**inputs):
    out, _ = _run(inputs)
    return out


# revision 23
# speedup vs baseline: 1.0237x; 1.0237x over previous
"""Trainium2 Bass kernel for nn_Attention (B=4, P=2048, D=768, H=12, hd=64).

Sharding: 8 cores = 4 batches x 2 half-head-groups (6 heads each).

Schedule design (v2): the Scalar engine's Exp stream (25.2M elems/core,
~1.44us per 1536-col unit, 132 units ~= 190us) is the critical resource.
Everything else is scheduled around keeping it saturated:
  - phase A computes only the qk features needed by the first chunks
    (ft0/ft3) cc-outer across 8 PSUM banks, gated by the input DMA, so
    the first Exp fires ~13us in (baseline: 55us).
  - all remaining PE work (leftover qk projection, v projection, AV,
    output projection) is emitted as fine-grained fill jobs in the PE
    slack between score matmuls, budgeted per exp unit.
  - score slabs are triple-buffered so AV may lag up to 2 chunks.
  - PSUM: 2x[128,1536] score units (6 banks) + shared 2-bank pool for
    all accumulation groups (AV / v / qk / proj), max 2 groups open.
  - v/proj PSUM evacuations run on the Pool (GpSimd) engine; DVE keeps
    the qk evacuations and the softmax normalization chain.
  - chunk order [p0q0 p0q1 p0q2 p1q0 p1q1 p2q0 p2q1 p1q2 p2q2 p0q3
    p1q3 p2q3] completes token blocks early so the output projection
    and its DMA overlap the exp stream; only tb3 remains in the tail.
  - output projection bias (b_proj/2) added on-chip by Pool; host sums
    the two half-head partials per batch and transposes.

Per-core layouts (host-prepared):
  xT   [769, 2048] bf16  rows 0..767 = x[b].T, row 768 = ones
  wqk  [768, 768]  bf16  [c, feat]; feat tiles q(01) q(23) q(45) k(01) k(23) k(45)
  wv   [769, 390]  bf16  [c(+bias/ones row), 6 heads x (64 v-dims, ones-col)]
  wp   [384, 768]  bf16  [feat (6 heads x 64), out-features]
  bqk  [128, 6]    f32   per-partition bias per qk feature tile
  bp   [128, 6]    f32   b_proj / 2 per out-feature tile
Output:
  yT   [768, 2048] f32   partial (pre pair-sum) transposed projection
"""

import sys
from collections import deque

import numpy as np

if "/opt/trn_rl_repo" not in sys.path:
    sys.path.insert(0, "/opt/trn_rl_repo")

B, P, D = 4, 2048, 768
H, HD = 12, 64
N_CORES = 8
H_LOC = 6
SCALE = HD ** -0.5

CC = 6          # contraction chunks of 128 over D=768
KT = 16         # k-position tiles of 128
PT = 16         # token tiles of 128
TB = 4          # token blocks of 512
VW = H_LOC * (HD + 1)  # 390, per-head wv cols [ones, v(64)]
VG = 128        # per-head vsb block: col0 denom-ones, cols 64:128 v
UNIT = 1536
NBLK = 2 * KT   # 32 512-col score blocks per chunk
TOTAL = NBLK * 512
N_UNITS = (TOTAL + UNIT - 1) // UNIT  # 11 (last = 1024)
PERIOD = 1437   # ns, measured steady exp-unit cadence
MM = 213        # ns, 512-col matmul at 2.4 GHz

# chunk order (pair, qq): early tb completion without needing late ft early
CHUNKS = [(0, 0), (0, 1), (0, 2), (1, 0), (1, 1), (2, 0), (2, 1),
          (1, 2), (2, 2), (0, 3), (1, 3), (2, 3)]

_PROG = None


def _build_program():
    import concourse.mybir as mybir
    import concourse.tile as tile
    from concourse import bacc

    f32 = mybir.dt.float32
    bf16 = mybir.dt.bfloat16
    AF = mybir.ActivationFunctionType

    nc = bacc.Bacc("TRN2")

    xT = nc.declare_dram_parameter("xT", [769, 2048], bf16, isOutput=False)
    wqk = nc.declare_dram_parameter("wqk", [768, 768], bf16, isOutput=False)
    wv = nc.declare_dram_parameter("wv", [769, VW], bf16, isOutput=False)
    wp = nc.declare_dram_parameter("wp", [384, 768], bf16, isOutput=False)
    bqk = nc.declare_dram_parameter("bqk", [128, 6], f32, isOutput=False)
    bp = nc.declare_dram_parameter("bp", [128, 6], f32, isOutput=False)
    yT = nc.declare_dram_parameter("yT", [768, 2048], f32, isOutput=True)

    with tile.TileContext(nc) as tc:
        with (
            tc.tile_pool(name="persist", bufs=1) as persist,
            tc.tile_pool(name="slabs", bufs=2) as slabs,
            tc.tile_pool(name="norm", bufs=3) as norm,
            tc.tile_pool(name="ysl", bufs=2) as yslp,
            tc.tile_pool(name="drs", bufs=4, space="DRAM") as drs,
        ):
            qkt = persist.tile([128, 6, 2048], bf16, tag="qkt")
            vsb = persist.tile([128, PT, H_LOC * VG], bf16, tag="vsb")
            otsb = persist.tile([128, 3, 2048], bf16, tag="otsb")
            bqk_sb = persist.tile([128, 6], f32, tag="bqk_sb")
            bp_sb = persist.tile([128, 6], f32, tag="bp_sb")
            wp_sb = persist.tile([128, 3, 768], bf16, tag="wp_sb")
            xts = [
                persist.tile([128 if i < CC else 1, 2048], bf16,
                             tag=f"xt{i}", name=f"xt{i}")
                for i in range(7)
            ]
            wqk_sbs = [
                persist.tile([128, 768], bf16, tag=f"wqk{i}", name=f"wqk{i}")
                for i in range(CC)
            ]
            wv_sbs = [
                persist.tile([128 if i < CC else 1, VW], bf16,
                             tag=f"wv{i}", name=f"wv{i}")
                for i in range(7)
            ]

            # ---- input DMA, priority ordered. Per-queue rings are FIFO;
            # rings run concurrently and share HBM bandwidth, so only the
            # phase-A-critical bytes go first: xT alone on the sync ring,
            # the ft0/ft3 columns of wqk (+biases) on the scalar ring.
            # wv rides the sync ring behind xT; the remaining wqk columns
            # and wp trail at the end of both rings.
            for ccx in range(CC):
                nc.sync.dma_start(out=xts[ccx],
                                  in_=xT[ccx * 128:(ccx + 1) * 128, :])
                for ft in (3, 0):
                    nc.gpsimd.dma_start(
                        out=wqk_sbs[ccx][:, ft * 128:(ft + 1) * 128],
                        in_=wqk[ccx * 128:(ccx + 1) * 128,
                                ft * 128:(ft + 1) * 128])
            nc.sync.dma_start(out=xts[6], in_=xT[768:769, :])
            nc.scalar.dma_start(out=bqk_sb, in_=bqk[:, :])
            for ccx in range(CC):
                nc.sync.dma_start(out=wv_sbs[ccx],
                                  in_=wv[ccx * 128:(ccx + 1) * 128, :])
            nc.sync.dma_start(out=wv_sbs[6], in_=wv[768:769, :])
            for ccx in range(CC):
                # remaining wqk columns: ft1+ft2 and ft4+ft5 are each
                # contiguous ranges, one transfer per pair per cc
                nc.sync.dma_start(
                    out=wqk_sbs[ccx][:, 128:384],
                    in_=wqk[ccx * 128:(ccx + 1) * 128, 128:384])
                nc.sync.dma_start(
                    out=wqk_sbs[ccx][:, 512:768],
                    in_=wqk[ccx * 128:(ccx + 1) * 128, 512:768])
            for fc in range(3):
                nc.sync.dma_start(out=wp_sb[:, fc, :],
                                  in_=wp[fc * 128:(fc + 1) * 128, :])
            nc.sync.dma_start(out=bp_sb, in_=bp[:, :])
            # zero the vsb gap columns on Pool, after its DMA triggers
            nc.gpsimd.memset(
                vsb.rearrange("p a (h g) -> p a h g", g=VG)[:, :, :, 1:64], 0.0)

            # pre-warm the exp ACT table during the DMA lead
            warmup = norm.tile([1, 1], f32, tag="warmup", bufs=1)
            nc.vector.memset(warmup, 0.0)
            nc.scalar.activation(out=warmup, in_=warmup, func=AF.Exp)

            # ===== phase A: qk projection for ft0 (q pair0) + ft3 (k pair0)
            # cc-outer over 8 PSUM groups, gated by per-cc DMA arrival.
            psA_ctx = tc.tile_pool(name="psA", bufs=8, space="PSUM")
            psA = psA_ctx.__enter__()
            qpA = {}
            for ft in (3, 0):
                for tb in range(TB):
                    qpA[(ft, tb)] = psA.tile([128, 512], f32, tag="qpA",
                                             name=f"qpA{ft}_{tb}")
            for ccx in range(CC):
                for ft in (3, 0):
                    for tb in range(TB):
                        nc.tensor.matmul(
                            qpA[(ft, tb)],
                            wqk_sbs[ccx][:, ft * 128:(ft + 1) * 128],
                            xts[ccx][:, tb * 512:(tb + 1) * 512],
                            start=(ccx == 0),
                            stop=(ccx == CC - 1),
                        )
            # evac order: what chunk-0's first exp units need first
            for ft, tb in ((3, 0), (0, 0), (3, 1), (3, 2), (3, 3),
                           (0, 1), (0, 2), (0, 3)):
                nc.vector.tensor_scalar_add(
                    out=qkt[:, ft, tb * 512:(tb + 1) * 512],
                    in0=qpA[(ft, tb)],
                    scalar1=bqk_sb[:, ft:ft + 1],
                )
            psA_ctx.__exit__(None, None, None)

            # ===== phase B: exp-gated chunk pipeline with fill scheduler
            with (
                tc.tile_pool(name="psum_s", bufs=2, space="PSUM") as psum_s,
                tc.tile_pool(name="psum_w", bufs=2, space="PSUM") as psum_w,
            ):
                # ---------- fill-job machinery ----------
                # a group = generator yielding (cost_ns, emit_fn) steps;
                # at most 2 groups hold psum_w tiles concurrently.
                # groups are picked by deadline (chunk index of first use).
                import heapq

                state = {
                    "budget": 0.0,
                    "unit": 0,
                    "v_evacs": 0,
                    "active": [],      # [(deadline, needs_v, gen), ...] <= 2
                    "queue": [],       # heap of (deadline, seq, needs_v, gen)
                    "seq": 0,
                    "pending": [],     # (ready_unit, emit_fn)
                    "norm_count": {},  # qq -> heads normalized
                    "proj_pushed": set(),
                    "chunk": 0,
                }

                def v_done():
                    return state["v_evacs"] >= PT

                def push(deadline, gen, needs_v=False):
                    heapq.heappush(
                        state["queue"],
                        (deadline, state["seq"], needs_v, gen))
                    state["seq"] += 1

                def v_group(pt):
                    vp = psum_w.tile([128, 512], f32, tag="grp",
                                     name=f"vp{pt}")
                    for ccx in range(7):
                        kk = 128 if ccx < CC else 1

                        def mm(ccx=ccx, kk=kk, vp=vp):
                            nc.tensor.matmul(
                                vp[:, 0:VW],
                                xts[ccx][0:kk, pt * 128:(pt + 1) * 128],
                                wv_sbs[ccx][0:kk, :],
                                start=(ccx == 0),
                                stop=(ccx == 6),
                            )
                        yield (162 if ccx < CC else 30, mm)

                    def evac(vp=vp):
                        vpv = vp[:, 0:VW].rearrange("p (h c) -> p h c", c=65)
                        vdst = vsb.rearrange(
                            "p a (h g) -> p a h g", g=VG)[:, pt]
                        nc.vector.tensor_copy(out=vdst[:, :, 0:1],
                                              in_=vpv[:, :, 0:1])
                        nc.vector.tensor_copy(out=vdst[:, :, 64:128],
                                              in_=vpv[:, :, 1:65])
                        state["v_evacs"] += 1
                    yield (0, evac)

                def qk_group(ft, tb):
                    qp = psum_w.tile([128, 512], f32, tag="grp",
                                     name=f"qp{ft}_{tb}")
                    for ccx in range(CC):
                        def mm(ccx=ccx, qp=qp):
                            nc.tensor.matmul(
                                qp,
                                wqk_sbs[ccx][:, ft * 128:(ft + 1) * 128],
                                xts[ccx][:, tb * 512:(tb + 1) * 512],
                                start=(ccx == 0),
                                stop=(ccx == CC - 1),
                            )
                        yield (MM, mm)

                    def evac(qp=qp):
                        nc.vector.tensor_scalar_add(
                            out=qkt[:, ft, tb * 512:(tb + 1) * 512],
                            in0=qp,
                            scalar1=bqk_sb[:, ft:ft + 1],
                        )
                    yield (0, evac)

                def norm_finish(ph, qq, osb, rb):
                    """emitted >=2 units after the AV group ends"""
                    pb = 64 * (ph % 2)
                    nc.vector.tensor_mul(
                        out=otsb[pb:pb + 64, ph // 2,
                                 qq * 512:(qq + 1) * 512],
                        in0=osb[64:128, :],
                        in1=rb[64:128, :],
                    )
                    cnt = state["norm_count"]
                    cnt[qq] = cnt.get(qq, 0) + 1
                    if cnt[qq] == H_LOC:
                        push_proj(qq)

                def av_group(p, qq, hd, slab):
                    ph = 2 * p + hd
                    op = psum_w.tile([128, 512], f32, tag="grp",
                                     name=f"op{ph}_{qq}")
                    for kc in range(KT):
                        def mm(kc=kc, op=op):
                            nc.tensor.matmul(
                                op,
                                vsb[:, kc, ph * VG:(ph + 1) * VG],
                                slab[:, kc * 2 + hd, :],
                                start=(kc == 0),
                                stop=(kc == KT - 1),
                            )
                        yield (MM, mm)

                    def drain(op=op):
                        osb = norm.tile([128, 512], f32, tag="osb")
                        nc.vector.tensor_copy(out=osb, in_=op)
                        rec = norm.tile([1, 512], f32, tag="rec", bufs=2)
                        rsc = norm.tile([1, 512], f32, tag="rsc", bufs=2)
                        nc.vector.reciprocal_approx_accurate(
                            out=rec, in_=osb[0:1, :], scratch=rsc)
                        dsc = drs.tile([1, 512], f32, tag="dsc")
                        nc.sync.dma_start(out=dsc, in_=rec)
                        rb = norm.tile([128, 512], f32, tag="rb", bufs=2)
                        nc.gpsimd.dma_start(
                            out=rb[64:128, :],
                            in_=dsc.partition_broadcast(64))
                        state["pending"].append(
                            (state["unit"] + 2,
                             lambda: norm_finish(ph, qq, osb, rb)))
                    yield (0, drain)

                def proj_group(of, tb):
                    pp = psum_w.tile([128, 512], f32, tag="grp",
                                     name=f"pp{of}_{tb}")
                    for fc in range(3):
                        def mm(fc=fc, pp=pp):
                            nc.tensor.matmul(
                                pp,
                                wp_sb[:, fc, of * 128:(of + 1) * 128],
                                otsb[:, fc, tb * 512:(tb + 1) * 512],
                                start=(fc == 0),
                                stop=(fc == 2),
                            )
                        yield (MM, mm)

                    def evac(pp=pp):
                        ysl = yslp.tile([128, 512], f32, tag="ysl")
                        nc.vector.tensor_scalar_add(
                            out=ysl, in0=pp, scalar1=bp_sb[:, of:of + 1])
                        nc.sync.dma_start(
                            out=yT[of * 128:(of + 1) * 128,
                                   tb * 512:(tb + 1) * 512],
                            in_=ysl,
                        )
                    yield (0, evac)

                def push_proj(tb):
                    if tb in state["proj_pushed"]:
                        return
                    state["proj_pushed"].add(tb)
                    for of in range(6):
                        push(state["chunk"] + 1.1, proj_group(of, tb))

                def refill_active():
                    while len(state["active"]) < 2 and state["queue"]:
                        dl, sq, needs_v, gen = state["queue"][0]
                        if needs_v and not v_done():
                            # find first runnable entry instead
                            runnable = [e for e in state["queue"]
                                        if not e[2]]
                            if not runnable:
                                return
                            entry = min(runnable)
                            state["queue"].remove(entry)
                            heapq.heapify(state["queue"])
                            dl, sq, needs_v, gen = entry
                        else:
                            heapq.heappop(state["queue"])
                        state["active"].append((dl, needs_v, gen))

                def emit_fills(limit=None):
                    emitted = 0
                    refill_active()
                    while state["active"] and state["budget"] > 0:
                        entry = state["active"].pop(0)
                        try:
                            cost, fn = next(entry[2])
                        except StopIteration:
                            refill_active()
                            continue
                        fn()
                        state["budget"] -= cost
                        state["active"].append(entry)
                        emitted += 1
                        if limit is not None and emitted >= limit:
                            break
                        refill_active()

                def force_drain(thr):
                    """fully emit every group whose deadline precedes thr —
                    consumers in chunk ceil(thr) are about to be emitted and
                    Tile only orders by program order."""
                    progress = True
                    while progress:
                        progress = False
                        for entry in list(state["active"]):
                            dl, needs_v, g = entry
                            if dl < thr and (not needs_v or v_done()):
                                state["active"].remove(entry)
                                for cost, fn in g:
                                    fn()
                                    state["budget"] -= cost
                                progress = True
                        while True:
                            cands = [e for e in state["queue"]
                                     if e[0] < thr
                                     and (not e[2] or v_done())]
                            if not cands:
                                break
                            entry = min(cands)
                            state["queue"].remove(entry)
                            heapq.heapify(state["queue"])
                            for cost, fn in entry[3]:
                                fn()
                                state["budget"] -= cost
                            progress = True

                def unit_tick():
                    u = state["unit"]
                    still = []
                    for ready, fn in state["pending"]:
                        if u >= ready:
                            fn()
                        else:
                            still.append((ready, fn))
                    state["pending"] = still
                    state["unit"] = u + 1

                # seed the queue: v projection, then leftover qk features
                # with deadlines = first chunk that reads them
                for pt in range(PT):
                    push(1.5 + 0.02 * pt, v_group(pt))
                first_pair_chunk = {0: 0, 1: 3, 2: 5}
                for ft, tb in ((4, 0), (4, 1), (4, 2), (4, 3),
                               (5, 0), (5, 1), (5, 2), (5, 3)):
                    # k-features: all four k-tiles are read within the
                    # pair's first chunk, so all must precede it
                    dl = first_pair_chunk[ft - 3] - 0.5 + 0.1 * tb
                    push(dl, qk_group(ft, tb))
                for ft in (1, 2):
                    for tb in range(TB):
                        dl = CHUNKS.index((ft, tb)) - 0.4
                        push(dl, qk_group(ft, tb))

                def score_mm(p, qq, sp, g, off):
                    kt, hd = g // 2, g % 2
                    pb = 64 * hd
                    qlo = qq * 512
                    nc.tensor.matmul(
                        sp[:, off:off + 512],
                        qkt[pb:pb + 64, 3 + p, kt * 128:(kt + 1) * 128],
                        qkt[pb:pb + 64, p, qlo:qlo + 512],
                        start=True,
                        stop=True,
                    )

                for ci, (p, qq) in enumerate(CHUNKS):
                    state["chunk"] = ci
                    slab = slabs.tile([128, NBLK, 512], bf16, tag="slab")
                    # PE below full p-state for the first chunks: charge
                    # emitted fill work at its real (slower) rate
                    slow = 1.3 if ci == 0 else (1.1 if ci == 1 else 1.0)
                    for u in range(N_UNITS):
                        unit_tick()
                        width = min(UNIT, TOTAL - u * UNIT)
                        nblk_u = width // 512
                        state["budget"] += (
                            PERIOD - ((nblk_u + 1) // 2) * MM * slow
                        ) / slow
                        if ci > 0 or u >= 4:
                            # chunk 0's first units are input-DMA gated;
                            # a v fill here would stall PE on the wv DMA
                            emit_fills()
                        # deadline enforcement, smeared across the chunk so
                        # it never dumps a multi-group blob before unit 0
                        force_drain(ci + (u + 1) / N_UNITS)
                        sp = psum_s.tile([128, UNIT], f32, tag="sp")
                        for j in range(nblk_u):
                            score_mm(p, qq, sp, u * 3 + j, j * 512)
                        nc.scalar.activation(
                            out=slab.rearrange("p a b -> p (a b)")[
                                :, u * UNIT:u * UNIT + width],
                            in_=sp[:, 0:width],
                            func=AF.Exp,
                            scale=SCALE,
                        )
                    # slab complete: queue its AV; must be fully emitted
                    # before chunk ci+2 reuses the slab buffer slot
                    for hd in range(2):
                        push(ci + 1.35, av_group(p, qq, hd, slab),
                             needs_v=True)

                # ---- tail: drain all remaining groups and pendings ----
                state["chunk"] = len(CHUNKS)
                state["budget"] = 1e9
                guard = 0
                while (state["active"] or state["queue"]
                       or state["pending"]):
                    unit_tick()
                    emit_fills(limit=64)
                    guard += 1
                    assert guard < 10000, "tail drain did not converge"

    nc.finalize()
    return nc


def _get_program():
    global _PROG
    if _PROG is None:
        _PROG = _build_program()
    return _PROG


def _prep_core_inputs(x, w_qkv, b_qkv, w_proj, b_proj, core):
    b, half = core // 2, core % 2
    heads = np.arange(H_LOC) + H_LOC * half
    d = np.arange(HD)

    import ml_dtypes
    bft = ml_dtypes.bfloat16
    xT = np.empty((769, 2048), bft)
    xT[:768] = x[b].T.astype(bft)
    xT[768] = 1.0

    # torch reshape quirk: feature (t, d, h) -> row t*768 + d*12 + h
    qk_rows = np.empty(768, np.int64)
    for j in range(3):
        for hp in range(2):
            hh = heads[2 * j + hp]
            base = j * 128 + hp * 64
            qk_rows[base:base + 64] = d * 12 + hh
            qk_rows[384 + base:384 + base + 64] = 768 + d * 12 + hh
    wqk = np.ascontiguousarray(w_qkv[qk_rows].T.astype(bft))
    bqk = np.ascontiguousarray(b_qkv[qk_rows].reshape(6, 128).T)

    # per-head cols: [64 v-dims, ones]; row 768 = bias (+1.0 in ones col)
    wv = np.zeros((769, VW), bft)
    for i in range(H_LOC):
        rows = 1536 + d * 12 + heads[i]
        wv[768, 65 * i] = 1.0
        wv[:768, 65 * i + 1:65 * i + 65] = w_qkv[rows].T.astype(bft)
        wv[768, 65 * i + 1:65 * i + 65] = b_qkv[rows]

    wp = np.empty((384, 768), bft)
    for i in range(H_LOC):
        cols = 64 * heads[i] + d
        wp[64 * i:64 * i + 64] = w_proj[:, cols].T
    bp = np.ascontiguousarray((b_proj * 0.5).reshape(6, 128).T)

    return {
        "xT": xT,
        "wqk": wqk,
        "wv": np.ascontiguousarray(wv),
        "wp": np.ascontiguousarray(wp),
        "bqk": bqk,
        "bp": np.ascontiguousarray(bp),
    }


def _run(inputs, trace=False, **kw):
    from concourse.bass_utils import run_bass_kernel_spmd

    nc = _get_program()
    x = np.asarray(inputs["x"], np.float32)
    w_qkv = np.asarray(inputs["w_qkv"], np.float32)
    b_qkv = np.asarray(inputs["b_qkv"], np.float32)
    w_proj = np.asarray(inputs["w_proj"], np.float32)
    b_proj = np.asarray(inputs["b_proj"], np.float32)

    in_maps = [
        _prep_core_inputs(x, w_qkv, b_qkv, w_proj, b_proj, c)
        for c in range(N_CORES)
    ]
    res = run_bass_kernel_spmd(nc, in_maps, list(range(N_CORES)),
                               trace=trace, **kw)

    out = np.empty((B, P, D), np.float32)
    for b in range(B):
        yt = res.results[2 * b]["yT"] + res.results[2 * b + 1]["yT"]
        out[b] = yt.T
    return out, res


def kernel(**inputs):
    out, _ = _run(inputs)
    return out
